# revision 9
# baseline (speedup 1.0000x reference)
"""Trainium2 Bass kernel for ExponentialKernelFiringRateModel (batched predict).

Model (reference semantics, fp32):
    v_t = (1-ds)*v_{t-1} + outer(I_t, a) + 1000*outer(f_{t-1}, b)      [B, N]
    z_t = v_t @ w ;  f_t = relu(100*tanh(poly((z_t - g_b)/1000)))      [B]
returns (pred_fs [T,B], vs [T,B,N]).

Strategy (8 cores, data-parallel over batch, 128 batch elements per core):
  The T=1000 serial scan is cut into 8 chunks of C=128 steps. Within a chunk
  z is affine in the chunk-local f history with weak coupling (|gamma|~0.01-
  0.1), so the nonlinear fixed point F = phi(Zbase + L F) is solved by a few
  Jacobi sweeps of tensor-engine matmuls ([C,C] Toeplitz operators); the
  sweep count NI comes from a fast host-side numpy rehearsal. The chunk-end
  state v_end (the only serial carry) is produced directly by three more
  matmuls, so the serial path never waits for the bulk pipeline. The bulk
  output vs is materialized per 64-step tile: tensor-engine matmuls expand
  u_t = a*I_t + 1000*b*f_{t-1} against block-diagonal constants (K=32 groups
  at legal PE quadrant bases), then one DVE `tensor_tensor_scan` per tile
  runs the IIR v = d*v + u in-place along the time axis (segment restarts
  via a zero-masked decay operand), and the result streams to HBM as flat
  contiguous tiles that the host reorders into [T,B,N].
"""

import functools
import numpy as np

T, B, N = 1000, 1024, 128
NCORES = 8
BLOC = B // NCORES          # 128 batch per core
C = 128                     # chunk length; last chunk is 104
NCH = 8
CLAST = T - (NCH - 1) * C   # 104
TT = 64                     # scan-tile length (2 tiles per chunk)
NTILES = NCH * 2
GQ = 32                     # u-matmul contraction group (PE quadrant size)

_last_results = None


# --------------------------------------------------------------------------
# host-side precompute (float64 -> float32 tiles)
# --------------------------------------------------------------------------

def _host_constants(ds, a, b, w, poly_coeff, g_b):
    f64 = np.float64
    d = (1.0 - ds).astype(f64)
    a64 = a.astype(f64)
    b64 = b.astype(f64)
    wf = w.reshape(-1).astype(f64)
    c2 = poly_coeff.astype(f64) ** 2
    gb = float(np.asarray(g_b))

    # cubic in raw z: p(z) = sum_k c2[k] ((z-gb)/1000)^k = e0+e1 z+e2 z^2+e3 z^3
    e = np.zeros(4, f64)
    powc = np.array([1.0])
    for k in range(4):
        e[: len(powc)] += c2[k] * powc
        powc = np.convolve(powc, [-gb / 1000.0, 1.0 / 1000.0])

    dp = np.empty((C + 1, N), f64)
    dp[0] = 1.0
    for m in range(1, C + 1):
        dp[m] = dp[m - 1] * d

    dw = dp * wf                      # dw[m, n] = d^m w
    tau = dw[:C] @ a64                # tau[m] = a . D^m w
    gam = 1000.0 * (dw[:C] @ b64)     # gam[m] = 1000 b . D^m w

    idx = np.arange(C)
    MK = idx[None, :] - idx[:, None]               # MK[j, k] = k - j
    TIT = np.where(MK >= 0, tau[np.clip(MK, 0, C - 1)], 0.0)      # [j, k]
    LT2 = np.where(MK >= 1, gam[np.clip(MK - 1, 0, C - 1)], 0.0)  # [i, k]
    GAMPAD = np.zeros((C, C), f64)
    GAMPAD[C - 1, :] = gam                          # row 127 -> gamma_k

    W2 = (dp[1:C + 1] * wf).T                       # [N, C] W2[n,k]=d^{k+1}w
    KI = dp[C - 1::-1] * a64                        # [C, N] KI[j]=a d^{C-1-j}
    KFp = 1000.0 * dp[C - 2::-1] * b64              # [C-1,N] KFp[i]=1000b d^{C-2-i}
    KF0PAD = np.zeros((C, N), f64)
    KF0PAD[C - 1, :] = 1000.0 * dp[C - 1] * b64
    DIAGC = np.diag(dp[C])                          # [N, N] diag(d^C)
    DIAGD = np.diag(d)                              # [N, N] diag(d)

    # u-matmul block constants: for group row s (0..31), column (nl, tl):
    #   RA32[s, nl*GQ + tl] = a_{n0+nl} * (s == tl), replicated at the four
    #   partition quadrants so lhsT/rhs share a legal base partition.
    ra = np.zeros((GQ, 16 * GQ, 8), f64)            # [s, (nl,tl), j]
    rb = np.zeros((GQ, 16 * GQ, 8), f64)
    for j in range(8):
        for nl in range(16):
            nn = 16 * j + nl
            for s in range(GQ):
                ra[s, nl * GQ + s, j] = a64[nn]
                rb[s, nl * GQ + s, j] = 1000.0 * b64[nn]
    RA32F = np.concatenate([np.concatenate([ra[:, :, j] for j in range(8)], 1)] * 4, 0)
    RB32F = np.concatenate([np.concatenate([rb[:, :, j] for j in range(8)], 1)] * 4, 0)

    mrow = np.repeat(d, TT)
    mrow[0::TT] = 0.0
    DMASK = np.tile(mrow, (128, 1))                 # [128, N*TT]
    DREP = np.tile(d, (128, 1))                     # [128, N]
    COLS = np.tile(np.array([e[3], e[2], e[1], e[0]], f64), (128, 1))

    cst = dict(W2=W2, TIT=TIT, LT2=LT2, GAMPAD=GAMPAD, KI=KI, KFp=KFp,
               KF0PAD=KF0PAD, DIAGC=DIAGC, DIAGD=DIAGD, RA32F=RA32F,
               RB32F=RB32F, DMASK=DMASK, DREP=DREP, COLS=COLS)
    cst = {k: np.ascontiguousarray(v, np.float32) for k, v in cst.items()}
    meta = dict(e=e, dp=dp, gam=gam, kf0=1000.0 * dp[C - 1] * b64)
    return cst, meta


def _pick_ni(Is, cst, meta):
    """Rehearse the chunked Jacobi iteration (numpy fp32); pick the first
    sweep count where successive sweeps differ by < 1e-3 absolute, plus one."""
    f32 = np.float32
    e0, e1, e2, e3 = (f32(x) for x in meta["e"])

    def phi(z):
        s = z * z
        return np.maximum(f32(100.0) * np.tanh(z * (s * e3 + e1) + (s * e2 + e0),
                                               dtype=f32), f32(0.0))

    W2 = cst["W2"]; TIT = cst["TIT"]; LT2 = cst["LT2"]
    KI = cst["KI"]; KFp = cst["KFp"]
    gam = meta["gam"].astype(f32)
    dC = meta["dp"][C].astype(f32)
    kf0 = meta["kf0"].astype(f32)          # 1000 * b * d^{C-1}

    need = 3
    vbT = np.zeros((N, B), f32)
    fb = np.zeros((B,), f32)
    for ci in range(NCH):
        Cc = C if ci < NCH - 1 else CLAST
        I_ch = Is[ci * C: ci * C + Cc].astype(f32)
        Zc = (W2.T[:Cc] @ vbT + TIT[:Cc, :Cc].T @ I_ch
              + gam[:Cc][:, None] * fb[None, :]).astype(f32)
        Fx = np.zeros((Cc, B), f32)
        used = 12
        for it in range(12):
            Fn = phi(Zc + LT2[:Cc, :Cc].T @ Fx)
            delta = float(np.abs(Fn - Fx).max())
            Fx = Fn
            if delta < 1e-3:
                used = it + 1
                break
        need = max(need, used)
        if ci < NCH - 1:
            vbT = (dC[:, None] * vbT + KI.T @ I_ch + KFp.T @ Fx[:C - 1]
                   + kf0[:, None] * fb[None, :]).astype(f32)
            fb = Fx[Cc - 1]
    return int(min(max(need + 1, 3), 12))


# --------------------------------------------------------------------------
# device program
# --------------------------------------------------------------------------

@functools.lru_cache(maxsize=4)
def _build(NI):
    import concourse.bacc as bacc
    import concourse.mybir as mybir
    from concourse.tile import TileContext

    f32 = mybir.dt.float32
    f32r = mybir.dt.float32r
    Op = mybir.AluOpType
    AF = mybir.ActivationFunctionType

    nc = bacc.Bacc(None, target_bir_lowering=False)

    def pin(name, shape):
        return nc.declare_dram_parameter(name, list(shape), f32, isOutput=False)

    IsS = pin("IsS", (T, BLOC))
    dW2 = pin("W2", (N, C)); dTIT = pin("TIT", (C, C)); dLT2 = pin("LT2", (C, C))
    dGAMPAD = pin("GAMPAD", (C, C)); dKI = pin("KI", (C, N))
    dKFp = pin("KFp", (C - 1, N)); dKF0PAD = pin("KF0PAD", (C, N))
    dDIAGC = pin("DIAGC", (N, N)); dDIAGD = pin("DIAGD", (N, N))
    dRA = nc.declare_dram_parameter("RA32F", [128, 4096], f32r, isOutput=False)
    dRB = nc.declare_dram_parameter("RB32F", [128, 4096], f32r, isOutput=False)
    dDMASK = pin("DMASK", (128, N * TT)); dDREP = pin("DREP", (128, N))
    dCOLS = pin("COLS", (128, 4))
    fs_out = nc.declare_dram_parameter("fs_dev", [NCH, C, BLOC], f32, isOutput=True)
    vs_out = nc.declare_dram_parameter("vs_dev", [NTILES, 128, N * TT], f32,
                                       isOutput=True)

    with TileContext(nc) as tc:
        with (
            tc.tile_pool(name="const", bufs=1) as cpool,
            tc.tile_pool(name="uv", bufs=3) as uvpool,
            tc.tile_pool(name="small", bufs=1) as spool,
            tc.tile_pool(name="ch", bufs=2) as chpool,
            tc.tile_pool(name="zp", bufs=2, space="PSUM") as zpool,
            tc.tile_pool(name="vp", bufs=1, space="PSUM") as vppool,
            tc.tile_pool(name="up", bufs=2, space="PSUM") as upool,
        ):
            def cload(dram, shape, tag, dt=f32):
                t = cpool.tile(list(shape), dt, tag=tag, name=tag)
                nc.sync.dma_start(t[:], dram[:])
                return t

            W2 = cload(dW2, (N, C), "cW2")
            TIT = cload(dTIT, (C, C), "cTIT")
            LT2 = cload(dLT2, (C, C), "cLT2")
            GAMPAD = cload(dGAMPAD, (C, C), "cGAMPAD")
            KI = cload(dKI, (C, N), "cKI")
            KFp = cload(dKFp, (C - 1, N), "cKFp")
            KF0PAD = cload(dKF0PAD, (C, N), "cKF0PAD")
            DIAGC = cload(dDIAGC, (N, N), "cDIAGC")
            DIAGD = cload(dDIAGD, (N, N), "cDIAGD")
            RA = cload(dRA, (128, 4096), "cRA", f32r)
            RB = cload(dRB, (128, 4096), "cRB", f32r)
            DMASK = cload(dDMASK, (128, N * TT), "cDMASK")
            DREP = cload(dDREP, (128, N), "cDREP")
            COLS = cload(dCOLS, (128, 4), "cCOLS")

            vbs = [spool.tile([N, BLOC], f32, tag="vba", name="vba"),
                   spool.tile([N, BLOC], f32, tag="vbb", name="vbb")]
            nc.vector.memset(vbs[0][:], 0.0)

            Fxs = []
            for ci in range(NCH):
                Cc = C if ci < NCH - 1 else CLAST
                t0 = ci * C
                It = spool.tile([128, BLOC], f32, tag=f"it{ci}")
                nc.gpsimd.dma_start(It[0:Cc, :], IsS[t0:t0 + Cc, :])
                It_r = spool.tile([128, BLOC], f32r, tag=f"itr{ci}", name=f"itr{ci}")
                nc.gpsimd.dma_start(It_r[0:Cc, :],
                                    IsS[t0:t0 + Cc, :].bitcast(f32r))
                Fx = spool.tile([128, BLOC], f32, tag=f"fx{ci}")
                nc.vector.memset(Fx[:], 0.0)
                vbc = vbs[ci % 2]
                Fprev = Fxs[-1] if ci > 0 else None

                # ---- serial chain: NI Jacobi sweeps ----
                for sw in range(NI):
                    z = zpool.tile([128, BLOC], f32, tag="z")
                    zs = z[0:Cc, :]
                    nc.tensor.matmul(zs, W2[:, 0:Cc], vbc[:], start=True, stop=False)
                    nc.tensor.matmul(zs, TIT[0:Cc, 0:Cc], It[0:Cc, :],
                                     start=False, stop=False)
                    if ci > 0:
                        nc.tensor.matmul(zs, GAMPAD[:, 0:Cc], Fprev[:],
                                         start=False, stop=False)
                    nc.tensor.matmul(zs, LT2[0:Cc, 0:Cc], Fx[0:Cc, :],
                                     start=False, stop=True)
                    S = chpool.tile([128, BLOC], f32, tag="S")
                    nc.scalar.activation(S[0:Cc, :], zs, AF.Square)
                    q = chpool.tile([128, BLOC], f32, tag="q")
                    nc.vector.tensor_scalar(q[0:Cc, :], S[0:Cc, :], COLS[0:Cc, 0:1],
                                            COLS[0:Cc, 2:3], op0=Op.mult, op1=Op.add)
                    m2 = chpool.tile([128, BLOC], f32, tag="m2")
                    nc.vector.scalar_tensor_tensor(m2[0:Cc, :], zs, 1.0, q[0:Cc, :],
                                                   op0=Op.mult, op1=Op.mult)
                    r = chpool.tile([128, BLOC], f32, tag="r")
                    nc.vector.scalar_tensor_tensor(r[0:Cc, :], S[0:Cc, :],
                                                   COLS[0:Cc, 1:2], m2[0:Cc, :],
                                                   op0=Op.mult, op1=Op.add)
                    Gt = chpool.tile([128, BLOC], f32, tag="G")
                    nc.scalar.activation(Gt[0:Cc, :], r[0:Cc, :], AF.Tanh,
                                         bias=COLS[0:Cc, 3:4])
                    nc.vector.tensor_scalar(Fx[0:Cc, :], Gt[0:Cc, :], 100.0, 0.0,
                                            op0=Op.mult, op1=Op.max)

                # ---- chunk-end state carry ----
                if ci < NCH - 1:
                    ve = vppool.tile([N, BLOC], f32, tag="ve")
                    nc.tensor.matmul(ve[:], DIAGC[:], vbc[:], start=True, stop=False)
                    nc.tensor.matmul(ve[:], KI[:], It[:], start=False, stop=False)
                    if ci > 0:
                        nc.tensor.matmul(ve[:], KF0PAD[:], Fprev[:],
                                         start=False, stop=False)
                    nc.tensor.matmul(ve[:], KFp[:], Fx[0:C - 1, :], start=False,
                                     stop=True)
                    nc.scalar.copy(vbs[(ci + 1) % 2][:], ve[:])

                nc.gpsimd.dma_start(fs_out[ci, 0:Cc, :], Fx[0:Cc, :])
                Fxs.append(Fx)

                # ---- bulk: u expansion + IIR scan + vs DMA ----
                # F_shift[k] = f_{t0+k-1}: row 0 = f_base, rows 1.. = Fx
                Fsh = spool.tile([128, BLOC], f32r, tag=f"fsh{ci % 3}")
                if ci == 0:
                    nc.vector.memset(Fsh[0:1, :].bitcast(f32), 0.0)
                else:
                    nc.gpsimd.dma_start(Fsh[0:1, :],
                                        Fprev[C - 1:C, :].bitcast(f32r))
                nsh = min(Cc, C - 1)
                nc.gpsimd.dma_start(Fsh[1:1 + nsh, :], Fx[0:nsh, :].bitcast(f32r))

                # d * v_base^T in [b, n] layout for the chunk-start injection
                if ci > 0:
                    vt = vppool.tile([128, N], f32, tag="vt")
                    nc.tensor.matmul(vt[:], vbc[:], DIAGD[:], start=True, stop=True)
                    vbTd = chpool.tile([128, N], f32, tag="vbTd")
                    nc.scalar.copy(vbTd[:], vt[:])

                # group 3 (rows 96..127) cannot sit at partition base 96
                # (PE quadrant bases are 0/32/64 only) -> stage at base 0.
                It96 = spool.tile([GQ, BLOC], f32r, tag=f"it96_{ci}",
                                  name=f"it96_{ci}")
                n96 = min(GQ, max(Cc - 3 * GQ, 0))
                if n96 > 0:
                    nc.gpsimd.dma_start(
                        It96[0:n96, :],
                        IsS[t0 + 3 * GQ:t0 + 3 * GQ + n96, :].bitcast(f32r))
                Fsh96 = spool.tile([GQ, BLOC], f32r, tag=f"fs96_{ci % 3}",
                                   name=f"fs96_{ci % 3}")
                nc.gpsimd.dma_start(Fsh96[0:GQ, :],
                                    Fx[3 * GQ - 1:4 * GQ - 1, :].bitcast(f32r))

                Uprev = None
                for h in range(2):
                    U = uvpool.tile([128, N * TT], f32, tag="uv")
                    U3 = U[:, :].rearrange("p (n t) -> p n t", n=N)
                    for g in (2 * h, 2 * h + 1):
                        tl0 = (g - 2 * h) * GQ          # 0 or 32 within tile
                        if g < 3:
                            lI = It_r[GQ * g:GQ * g + GQ, :]
                            lF = Fsh[GQ * g:GQ * g + GQ, :]
                            rb0 = GQ * g
                        else:
                            lI = It96[0:GQ, :]
                            lF = Fsh96[0:GQ, :]
                            rb0 = 0
                        for pj in range(4):             # pairs of n-slices
                            ups = upool.tile([128, 1024], f32, tag="up")
                            for half in range(2):
                                j = 2 * pj + half
                                seg = ups[:, half * 512:half * 512 + 512]
                                nc.tensor.matmul(
                                    seg, lI,
                                    RA[rb0:rb0 + GQ, j * 512:(j + 1) * 512],
                                    start=True, stop=False)
                                nc.tensor.matmul(
                                    seg, lF,
                                    RB[rb0:rb0 + GQ, j * 512:(j + 1) * 512],
                                    start=False, stop=True)
                            src = ups[:, :].rearrange("p (n t) -> p n t", n=GQ)
                            nc.scalar.copy(
                                U3[:, 32 * pj:32 * pj + 32, tl0:tl0 + GQ], src)
                    if h == 0:
                        if ci > 0:
                            nc.vector.tensor_tensor(U[:, 0::TT], U[:, 0::TT],
                                                    vbTd[:], Op.add)
                    else:
                        tmp = chpool.tile([128, N], f32, tag="inj")
                        nc.vector.tensor_tensor(tmp[:], Uprev[:, TT - 1::TT],
                                                DREP[:], Op.mult)
                        nc.vector.tensor_tensor(U[:, 0::TT], U[:, 0::TT], tmp[:],
                                                Op.add)
                    nc.vector.tensor_tensor_scan(U[:, :], DMASK[:, :], U[:, :],
                                                 initial=0.0, op0=Op.mult,
                                                 op1=Op.add)
                    nc.sync.dma_start(vs_out[2 * ci + h, :, :], U[:, :])
                    Uprev = U

    nc.compile()
    return nc


# --------------------------------------------------------------------------
# entry point
# --------------------------------------------------------------------------

def _ensure_axon_hooks_shim():
    """bass_utils' axon trace path imports antenv.axon_hooks, which may be
    missing from this image; give it a settable no-op shim so trace=True
    degrades gracefully instead of raising."""
    import sys
    import types
    try:
        import antenv.axon_hooks  # noqa: F401
        return sys.modules["antenv.axon_hooks"]
    except ImportError:
        pass
    import antenv
    mod = types.ModuleType("antenv.axon_hooks")
    mod._hook = None
    mod.set_axon_ntff_profile_hook = lambda h: setattr(mod, "_hook", h)
    mod.get_axon_ntff_profile_hook = lambda: mod._hook
    sys.modules["antenv.axon_hooks"] = mod
    antenv.axon_hooks = mod
    return mod


def kernel(Is, ds, a, b, w, poly_coeff, g_b):
    global _last_results
    import os
    from concourse.bass_utils import run_bass_kernel_spmd

    _ensure_axon_hooks_shim()

    Is = np.ascontiguousarray(np.asarray(Is, np.float32))
    cst, meta = _host_constants(np.asarray(ds), np.asarray(a), np.asarray(b),
                                np.asarray(w), np.asarray(poly_coeff),
                                np.asarray(g_b))
    NI = _pick_ni(Is, cst, meta)

    nc = _build(NI)

    in_maps = []
    for c in range(NCORES):
        m = dict(cst)
        m["IsS"] = np.ascontiguousarray(Is[:, c * BLOC:(c + 1) * BLOC])
        in_maps.append(m)

    trace = os.environ.get("BASS_TRACE", "").lower() in ("1", "true", "yes")
    res = run_bass_kernel_spmd(nc, in_maps, list(range(NCORES)), trace=trace)
    _last_results = res

    fs = np.empty((T, B), np.float32)
    vs = np.empty((T, B, N), np.float32)
    for c in range(NCORES):
        out = res.results[c]
        fsd = out["fs_dev"]
        vsd = out["vs_dev"].reshape(NTILES, 128, N, TT)
        b0 = c * BLOC
        for ci in range(NCH):
            Cc = C if ci < NCH - 1 else CLAST
            fs[ci * C:ci * C + Cc, b0:b0 + BLOC] = fsd[ci, :Cc]
            for h in range(2):
                ntt = min(TT, Cc - h * TT)
                if ntt <= 0:
                    continue
                tt0 = ci * C + h * TT
                vs[tt0:tt0 + ntt, b0:b0 + BLOC, :] = (
                    vsd[2 * ci + h, :, :, :ntt].transpose(2, 0, 1))
    return fs, vs


# revision 11
# speedup vs baseline: 1.1772x; 1.1772x over previous
"""Trainium2 Bass kernel for ExponentialKernelFiringRateModel (batched predict).

Model (reference semantics, fp32):
    v_t = (1-ds)*v_{t-1} + outer(I_t, a) + 1000*outer(f_{t-1}, b)      [B, N]
    z_t = v_t @ w ;  f_t = relu(100*tanh(poly((z_t - g_b)/1000)))      [B]
returns (pred_fs [T,B], vs [T,B,N]).

Strategy (8 cores, data-parallel over batch, 128 batch elements per core):
  The T=1000 serial scan is cut into 8 chunks of C=128 steps. Within a chunk
  z is affine in the chunk-local f history with weak coupling (|gamma|~0.01-
  0.1), so the nonlinear fixed point F = phi(Zbase + L F) is solved by a few
  Jacobi sweeps of tensor-engine matmuls ([C,C] Toeplitz operators); the
  sweep count NI comes from a fast host-side numpy rehearsal. The chunk-end
  state v_end (the only serial carry) is produced directly by three more
  matmuls, so the serial path never waits for the bulk pipeline. The bulk
  output vs is materialized per 64-step tile: tensor-engine matmuls expand
  u_t = a*I_t + 1000*b*f_{t-1} against block-diagonal constants (K=32 groups
  at legal PE quadrant bases), then one DVE `tensor_tensor_scan` per tile
  runs the IIR v = d*v + u in-place along the time axis (segment restarts
  via a zero-masked decay operand), and the result streams to HBM as flat
  contiguous tiles that the host reorders into [T,B,N].
"""

import functools
import numpy as np

T, B, N = 1000, 1024, 128
NCORES = 8
BLOC = B // NCORES          # 128 batch per core
C = 128                     # chunk length; last chunk is 104
NCH = 8
CLAST = T - (NCH - 1) * C   # 104
TT = 64                     # scan-tile length (2 tiles per chunk)
NTILES = NCH * 2
GQ = 32                     # u-matmul contraction group (PE quadrant size)

_last_results = None


# --------------------------------------------------------------------------
# host-side precompute (float64 -> float32 tiles)
# --------------------------------------------------------------------------

def _host_constants(ds, a, b, w, poly_coeff, g_b):
    f64 = np.float64
    d = (1.0 - ds).astype(f64)
    a64 = a.astype(f64)
    b64 = b.astype(f64)
    wf = w.reshape(-1).astype(f64)
    c2 = poly_coeff.astype(f64) ** 2
    gb = float(np.asarray(g_b))

    # cubic in raw z: p(z) = sum_k c2[k] ((z-gb)/1000)^k = e0+e1 z+e2 z^2+e3 z^3
    e = np.zeros(4, f64)
    powc = np.array([1.0])
    for k in range(4):
        e[: len(powc)] += c2[k] * powc
        powc = np.convolve(powc, [-gb / 1000.0, 1.0 / 1000.0])

    dp = np.empty((C + 1, N), f64)
    dp[0] = 1.0
    for m in range(1, C + 1):
        dp[m] = dp[m - 1] * d

    dw = dp * wf                      # dw[m, n] = d^m w
    tau = dw[:C] @ a64                # tau[m] = a . D^m w
    gam = 1000.0 * (dw[:C] @ b64)     # gam[m] = 1000 b . D^m w

    idx = np.arange(C)
    MK = idx[None, :] - idx[:, None]               # MK[j, k] = k - j
    TIT = np.where(MK >= 0, tau[np.clip(MK, 0, C - 1)], 0.0)      # [j, k]
    LT2 = np.where(MK >= 1, gam[np.clip(MK - 1, 0, C - 1)], 0.0)  # [i, k]
    GAMPAD = np.zeros((C, C), f64)
    GAMPAD[C - 1, :] = gam                          # row 127 -> gamma_k

    W2 = (dp[1:C + 1] * wf).T                       # [N, C] W2[n,k]=d^{k+1}w
    KI = dp[C - 1::-1] * a64                        # [C, N] KI[j]=a d^{C-1-j}
    KFp = 1000.0 * dp[C - 2::-1] * b64              # [C-1,N] KFp[i]=1000b d^{C-2-i}
    KF0PAD = np.zeros((C, N), f64)
    KF0PAD[C - 1, :] = 1000.0 * dp[C - 1] * b64
    DIAGC = np.diag(dp[C])                          # [N, N] diag(d^C)
    DIAGD = np.diag(d)                              # [N, N] diag(d)

    # u-matmul block constant: one K=64 operand per group.  lhsT rows 0..31
    # hold I_t, rows 32..63 hold f_{t-1}; RAB[s, j*512 + nl*32 + tl] pairs
    # row s with output column (n0+nl, tl): a_n delta(s,tl) on the I half,
    # 1000 b_n delta(s-32,tl) on the f half.
    RAB = np.zeros((2 * GQ, 4096), f64)
    for j in range(8):
        for nl in range(16):
            nn = 16 * j + nl
            for s in range(GQ):
                RAB[s, j * 512 + nl * GQ + s] = a64[nn]
                RAB[GQ + s, j * 512 + nl * GQ + s] = 1000.0 * b64[nn]

    mrow = np.repeat(d, TT)
    mrow[0::TT] = 0.0
    DMASK = np.tile(mrow, (128, 1))                 # [128, N*TT]
    DREP = np.tile(d, (128, 1))                     # [128, N]
    COLS = np.tile(np.array([e[3], e[2], e[1], e[0]], f64), (128, 1))

    cst = dict(W2=W2, TIT=TIT, LT2=LT2, GAMPAD=GAMPAD, KI=KI, KFp=KFp,
               KF0PAD=KF0PAD, DIAGC=DIAGC, DIAGD=DIAGD, RAB=RAB,
               DMASK=DMASK, DREP=DREP, COLS=COLS)
    cst = {k: np.ascontiguousarray(v, np.float32) for k, v in cst.items()}
    meta = dict(e=e, dp=dp, gam=gam, kf0=1000.0 * dp[C - 1] * b64)
    return cst, meta


def _pick_ni(Is, cst, meta):
    """Rehearse the chunked Jacobi iteration (numpy fp32); pick the first
    sweep count where successive sweeps differ by < 1e-3 absolute, plus one."""
    f32 = np.float32
    e0, e1, e2, e3 = (f32(x) for x in meta["e"])

    def phi(z):
        s = z * z
        return np.maximum(f32(100.0) * np.tanh(z * (s * e3 + e1) + (s * e2 + e0),
                                               dtype=f32), f32(0.0))

    W2 = cst["W2"]; TIT = cst["TIT"]; LT2 = cst["LT2"]
    KI = cst["KI"]; KFp = cst["KFp"]
    gam = meta["gam"].astype(f32)
    dC = meta["dp"][C].astype(f32)
    kf0 = meta["kf0"].astype(f32)          # 1000 * b * d^{C-1}

    need = 3
    vbT = np.zeros((N, B), f32)
    fb = np.zeros((B,), f32)
    for ci in range(NCH):
        Cc = C if ci < NCH - 1 else CLAST
        I_ch = Is[ci * C: ci * C + Cc].astype(f32)
        Zc = (W2.T[:Cc] @ vbT + TIT[:Cc, :Cc].T @ I_ch
              + gam[:Cc][:, None] * fb[None, :]).astype(f32)
        Fx = np.zeros((Cc, B), f32)
        used = 12
        for it in range(12):
            Fn = phi(Zc + LT2[:Cc, :Cc].T @ Fx)
            delta = float(np.abs(Fn - Fx).max())
            Fx = Fn
            if delta < 5e-3:
                used = it + 1
                break
        need = max(need, used)
        if ci < NCH - 1:
            vbT = (dC[:, None] * vbT + KI.T @ I_ch + KFp.T @ Fx[:C - 1]
                   + kf0[:, None] * fb[None, :]).astype(f32)
            fb = Fx[Cc - 1]
    return int(min(max(need + 1, 3), 12))


# --------------------------------------------------------------------------
# device program
# --------------------------------------------------------------------------

@functools.lru_cache(maxsize=4)
def _build(NI):
    import concourse.bacc as bacc
    import concourse.mybir as mybir
    from concourse.tile import TileContext

    f32 = mybir.dt.float32
    f32r = mybir.dt.float32r
    Op = mybir.AluOpType
    AF = mybir.ActivationFunctionType

    nc = bacc.Bacc(None, target_bir_lowering=False)

    def pin(name, shape, dt=f32):
        return nc.declare_dram_parameter(name, list(shape), dt, isOutput=False)

    IsS = pin("IsS", (T, BLOC))
    dW2 = pin("W2", (N, C), f32r); dTIT = pin("TIT", (C, C), f32r)
    dLT2 = pin("LT2", (C, C), f32r); dGAMPAD = pin("GAMPAD", (C, C), f32r)
    dKI = pin("KI", (C, N), f32r); dKFp = pin("KFp", (C - 1, N), f32r)
    dKF0PAD = pin("KF0PAD", (C, N), f32r); dDIAGC = pin("DIAGC", (N, N), f32r)
    dDIAGD = pin("DIAGD", (N, N), f32r)
    dRAB = pin("RAB", (2 * GQ, 4096), f32r)
    dDMASK = pin("DMASK", (128, N * TT)); dDREP = pin("DREP", (128, N))
    dCOLS = pin("COLS", (128, 4))
    fs_out = nc.declare_dram_parameter("fs_dev", [NCH, C, BLOC], f32, isOutput=True)
    vs_out = nc.declare_dram_parameter("vs_dev", [NTILES, 128, N * TT], f32,
                                       isOutput=True)

    with TileContext(nc) as tc:
        with (
            tc.tile_pool(name="const", bufs=1) as cpool,
            tc.tile_pool(name="uv", bufs=3) as uvpool,
            tc.tile_pool(name="small", bufs=1) as spool,
            tc.tile_pool(name="ch", bufs=2) as chpool,
            tc.tile_pool(name="zp", bufs=2, space="PSUM") as zpool,
            tc.tile_pool(name="vp", bufs=1, space="PSUM") as vppool,
            tc.tile_pool(name="up", bufs=2, space="PSUM") as upool,
        ):
            def cload(dram, shape, tag, dt=f32):
                t = cpool.tile(list(shape), dt, tag=tag, name=tag)
                nc.sync.dma_start(t[:], dram[:])
                return t

            W2 = cload(dW2, (N, C), "cW2", f32r)
            TIT = cload(dTIT, (C, C), "cTIT", f32r)
            LT2 = cload(dLT2, (C, C), "cLT2", f32r)
            GAMPAD = cload(dGAMPAD, (C, C), "cGAMPAD", f32r)
            KI = cload(dKI, (C, N), "cKI", f32r)
            KFp = cload(dKFp, (C - 1, N), "cKFp", f32r)
            KF0PAD = cload(dKF0PAD, (C, N), "cKF0PAD", f32r)
            DIAGC = cload(dDIAGC, (N, N), "cDIAGC", f32r)
            DIAGD = cload(dDIAGD, (N, N), "cDIAGD", f32r)
            RAB = cload(dRAB, (2 * GQ, 4096), "cRAB", f32r)
            DMASK = cload(dDMASK, (128, N * TT), "cDMASK")
            DREP = cload(dDREP, (128, N), "cDREP")
            COLS = cload(dCOLS, (128, 4), "cCOLS")

            vbs = [spool.tile([N, BLOC], f32r, tag="vba", name="vba"),
                   spool.tile([N, BLOC], f32r, tag="vbb", name="vbb")]
            nc.vector.memset(vbs[0][:].bitcast(f32), 0.0)

            Fxs = []
            for ci in range(NCH):
                Cc = C if ci < NCH - 1 else CLAST
                t0 = ci * C
                It = spool.tile([128, BLOC], f32r, tag=f"it{ci}")
                nc.gpsimd.dma_start(It[0:Cc, :], IsS[t0:t0 + Cc, :].bitcast(f32r))
                Fx = spool.tile([128, BLOC], f32r, tag=f"fx{ci}")
                nc.vector.memset(Fx[:].bitcast(f32), 0.0)
                vbc = vbs[ci % 2]
                Fprev = Fxs[-1] if ci > 0 else None

                # ---- serial chain: NI Jacobi sweeps ----
                for sw in range(NI):
                    z = zpool.tile([128, BLOC], f32, tag="z")
                    zs = z[0:Cc, :]
                    nc.tensor.matmul(zs, W2[:, 0:Cc], vbc[:], start=True, stop=False)
                    nc.tensor.matmul(zs, TIT[0:Cc, 0:Cc], It[0:Cc, :],
                                     start=False, stop=False)
                    if ci > 0:
                        nc.tensor.matmul(zs, GAMPAD[:, 0:Cc], Fprev[:],
                                         start=False, stop=False)
                    nc.tensor.matmul(zs, LT2[0:Cc, 0:Cc], Fx[0:Cc, :],
                                     start=False, stop=True)
                    S = chpool.tile([128, BLOC], f32, tag="S")
                    nc.scalar.activation(S[0:Cc, :], zs, AF.Square)
                    q = chpool.tile([128, BLOC], f32, tag="q")
                    nc.vector.tensor_scalar(q[0:Cc, :], S[0:Cc, :], COLS[0:Cc, 0:1],
                                            COLS[0:Cc, 2:3], op0=Op.mult, op1=Op.add)
                    m2 = chpool.tile([128, BLOC], f32, tag="m2")
                    nc.vector.scalar_tensor_tensor(m2[0:Cc, :], zs, 1.0, q[0:Cc, :],
                                                   op0=Op.mult, op1=Op.mult)
                    r = chpool.tile([128, BLOC], f32, tag="r")
                    nc.vector.scalar_tensor_tensor(r[0:Cc, :], S[0:Cc, :],
                                                   COLS[0:Cc, 1:2], m2[0:Cc, :],
                                                   op0=Op.mult, op1=Op.add)
                    Gt = chpool.tile([128, BLOC], f32, tag="G")
                    nc.scalar.activation(Gt[0:Cc, :], r[0:Cc, :], AF.Tanh,
                                         bias=COLS[0:Cc, 3:4])
                    nc.vector.tensor_scalar(Fx[0:Cc, :], Gt[0:Cc, :], 100.0, 0.0,
                                            op0=Op.mult, op1=Op.max)

                # ---- chunk-end state carry ----
                if ci < NCH - 1:
                    ve = vppool.tile([N, BLOC], f32, tag="ve")
                    nc.tensor.matmul(ve[:], DIAGC[:], vbc[:], start=True, stop=False)
                    nc.tensor.matmul(ve[:], KI[:], It[:], start=False, stop=False)
                    if ci > 0:
                        nc.tensor.matmul(ve[:], KF0PAD[:], Fprev[:],
                                         start=False, stop=False)
                    nc.tensor.matmul(ve[:], KFp[:], Fx[0:C - 1, :], start=False,
                                     stop=True)
                    nc.scalar.copy(vbs[(ci + 1) % 2][:], ve[:])

                nc.gpsimd.dma_start(fs_out[ci, 0:Cc, :], Fx[0:Cc, :].bitcast(f32))
                Fxs.append(Fx)

                # ---- combined u operands: rows 0..31 = I, rows 32..63 = f_(t-1)
                cmbs = []
                for g in range(4):
                    cmb = spool.tile([2 * GQ, BLOC], f32r, tag=f"cmb{g}_{ci % 3}",
                                     name=f"cmb{g}_{ci % 3}")
                    ni_ = min(GQ, max(Cc - GQ * g, 0))
                    if ni_ > 0:
                        nc.gpsimd.dma_start(
                            cmb[0:ni_, :],
                            IsS[t0 + GQ * g:t0 + GQ * g + ni_, :].bitcast(f32r))
                    if g == 0:
                        if ci == 0:
                            nc.vector.memset(cmb[GQ:GQ + 1, :].bitcast(f32), 0.0)
                        else:
                            nc.gpsimd.dma_start(cmb[GQ:GQ + 1, :],
                                                Fprev[C - 1:C, :])
                        nc.gpsimd.dma_start(cmb[GQ + 1:2 * GQ, :], Fx[0:GQ - 1, :])
                    else:
                        nc.gpsimd.dma_start(cmb[GQ:2 * GQ, :],
                                            Fx[GQ * g - 1:GQ * g + GQ - 1, :])
                    cmbs.append(cmb)

                # d * v_base^T in [b, n] layout for the chunk-start injection
                if ci > 0:
                    vt = vppool.tile([128, N], f32, tag="vt")
                    nc.tensor.matmul(vt[:], vbc[:], DIAGD[:], start=True, stop=True)
                    vbTd = chpool.tile([128, N], f32, tag="vbTd")
                    nc.scalar.copy(vbTd[:], vt[:])

                Uprev = None
                for h in range(2):
                    U = uvpool.tile([128, N * TT], f32, tag="uv")
                    U3 = U[:, :].rearrange("p (n t) -> p n t", n=N)
                    for g in (2 * h, 2 * h + 1):
                        tl0 = (g - 2 * h) * GQ          # 0 or 32 within tile
                        for pj in range(4):             # pairs of n-slices
                            ups = upool.tile([128, 1024], f32, tag="up")
                            for half in range(2):
                                j = 2 * pj + half
                                seg = ups[:, half * 512:half * 512 + 512]
                                nc.tensor.matmul(
                                    seg, cmbs[g][:],
                                    RAB[:, j * 512:(j + 1) * 512],
                                    start=True, stop=True)
                            src = ups[:, :].rearrange("p (n t) -> p n t", n=GQ)
                            nc.scalar.copy(
                                U3[:, 32 * pj:32 * pj + 32, tl0:tl0 + GQ], src)
                    if h == 0:
                        if ci > 0:
                            nc.vector.tensor_tensor(U[:, 0::TT], U[:, 0::TT],
                                                    vbTd[:], Op.add)
                    else:
                        tmp = chpool.tile([128, N], f32, tag="inj")
                        nc.vector.tensor_tensor(tmp[:], Uprev[:, TT - 1::TT],
                                                DREP[:], Op.mult)
                        nc.vector.tensor_tensor(U[:, 0::TT], U[:, 0::TT], tmp[:],
                                                Op.add)
                    nc.vector.tensor_tensor_scan(U[:, :], DMASK[:, :], U[:, :],
                                                 initial=0.0, op0=Op.mult,
                                                 op1=Op.add)
                    nc.sync.dma_start(vs_out[2 * ci + h, :, :], U[:, :])
                    Uprev = U

    nc.compile()
    return nc


# --------------------------------------------------------------------------
# entry point
# --------------------------------------------------------------------------

def _ensure_axon_hooks_shim():
    """bass_utils' axon trace path imports antenv.axon_hooks, which may be
    missing from this image; give it a settable no-op shim so trace=True
    degrades gracefully instead of raising."""
    import sys
    import types
    try:
        import antenv.axon_hooks  # noqa: F401
        return sys.modules["antenv.axon_hooks"]
    except ImportError:
        pass
    import antenv
    mod = types.ModuleType("antenv.axon_hooks")
    mod._hook = None
    mod.set_axon_ntff_profile_hook = lambda h: setattr(mod, "_hook", h)
    mod.get_axon_ntff_profile_hook = lambda: mod._hook
    sys.modules["antenv.axon_hooks"] = mod
    antenv.axon_hooks = mod
    return mod


def kernel(Is, ds, a, b, w, poly_coeff, g_b):
    global _last_results
    import os
    from concourse.bass_utils import run_bass_kernel_spmd

    _ensure_axon_hooks_shim()

    Is = np.ascontiguousarray(np.asarray(Is, np.float32))
    cst, meta = _host_constants(np.asarray(ds), np.asarray(a), np.asarray(b),
                                np.asarray(w), np.asarray(poly_coeff),
                                np.asarray(g_b))
    NI = _pick_ni(Is, cst, meta)

    nc = _build(NI)

    in_maps = []
    for c in range(NCORES):
        m = dict(cst)
        m["IsS"] = np.ascontiguousarray(Is[:, c * BLOC:(c + 1) * BLOC])
        in_maps.append(m)

    trace = os.environ.get("BASS_TRACE", "").lower() in ("1", "true", "yes")
    res = run_bass_kernel_spmd(nc, in_maps, list(range(NCORES)), trace=trace)
    _last_results = res

    fs = np.empty((T, B), np.float32)
    vs = np.empty((T, B, N), np.float32)
    for c in range(NCORES):
        out = res.results[c]
        fsd = out["fs_dev"]
        vsd = out["vs_dev"].reshape(NTILES, 128, N, TT)
        b0 = c * BLOC
        for ci in range(NCH):
            Cc = C if ci < NCH - 1 else CLAST
            fs[ci * C:ci * C + Cc, b0:b0 + BLOC] = fsd[ci, :Cc]
            for h in range(2):
                ntt = min(TT, Cc - h * TT)
                if ntt <= 0:
                    continue
                tt0 = ci * C + h * TT
                vs[tt0:tt0 + ntt, b0:b0 + BLOC, :] = (
                    vsd[2 * ci + h, :, :, :ntt].transpose(2, 0, 1))
    return fs, vs


# revision 14
# speedup vs baseline: 1.1873x; 1.0085x over previous
"""Trainium2 Bass kernel for ExponentialKernelFiringRateModel (batched predict).

Model (reference semantics, fp32):
    v_t = (1-ds)*v_{t-1} + outer(I_t, a) + 1000*outer(f_{t-1}, b)      [B, N]
    z_t = v_t @ w ;  f_t = relu(100*tanh(poly((z_t - g_b)/1000)))      [B]
returns (pred_fs [T,B], vs [T,B,N]).

Strategy (8 cores, data-parallel over batch, 128 batch elements per core):
  The T=1000 serial scan is cut into 8 chunks of C=128 steps. Within a chunk
  z is affine in the chunk-local f history with weak coupling (|gamma|~0.01-
  0.1), so the nonlinear fixed point F = phi(Zbase + L F) is solved by a few
  Jacobi sweeps of tensor-engine matmuls ([C,C] Toeplitz operators); the
  sweep count NI comes from a fast host-side numpy rehearsal. The chunk-end
  state v_end (the only serial carry) is produced directly by three more
  matmuls, so the serial path never waits for the bulk pipeline. The bulk
  output vs is materialized per 64-step tile: tensor-engine matmuls expand
  u_t = a*I_t + 1000*b*f_{t-1} against block-diagonal constants (K=32 groups
  at legal PE quadrant bases), then one DVE `tensor_tensor_scan` per tile
  runs the IIR v = d*v + u in-place along the time axis (segment restarts
  via a zero-masked decay operand), and the result streams to HBM as flat
  contiguous tiles that the host reorders into [T,B,N].
"""

import functools
import numpy as np

T, B, N = 1000, 1024, 128
NCORES = 8
BLOC = B // NCORES          # 128 batch per core
C = 128                     # chunk length; last chunk is 104
NCH = 8
CLAST = T - (NCH - 1) * C   # 104
TT = 64                     # scan-tile length (2 tiles per chunk)
NTILES = NCH * 2
GQ = 32                     # u-matmul contraction group (PE quadrant size)

_last_results = None


# --------------------------------------------------------------------------
# host-side precompute (float64 -> float32 tiles)
# --------------------------------------------------------------------------

def _host_constants(ds, a, b, w, poly_coeff, g_b):
    f64 = np.float64
    d = (1.0 - ds).astype(f64)
    a64 = a.astype(f64)
    b64 = b.astype(f64)
    wf = w.reshape(-1).astype(f64)
    c2 = poly_coeff.astype(f64) ** 2
    gb = float(np.asarray(g_b))

    # cubic in raw z: p(z) = sum_k c2[k] ((z-gb)/1000)^k = e0+e1 z+e2 z^2+e3 z^3
    e = np.zeros(4, f64)
    powc = np.array([1.0])
    for k in range(4):
        e[: len(powc)] += c2[k] * powc
        powc = np.convolve(powc, [-gb / 1000.0, 1.0 / 1000.0])

    dp = np.empty((C + 1, N), f64)
    dp[0] = 1.0
    for m in range(1, C + 1):
        dp[m] = dp[m - 1] * d

    dw = dp * wf                      # dw[m, n] = d^m w
    tau = dw[:C] @ a64                # tau[m] = a . D^m w
    gam = 1000.0 * (dw[:C] @ b64)     # gam[m] = 1000 b . D^m w

    idx = np.arange(C)
    MK = idx[None, :] - idx[:, None]               # MK[j, k] = k - j
    TIT = np.where(MK >= 0, tau[np.clip(MK, 0, C - 1)], 0.0)      # [j, k]
    LT2 = np.where(MK >= 1, gam[np.clip(MK - 1, 0, C - 1)], 0.0)  # [i, k]
    GAMPAD = np.zeros((C, C), f64)
    GAMPAD[C - 1, :] = gam                          # row 127 -> gamma_k

    W2 = (dp[1:C + 1] * wf).T                       # [N, C] W2[n,k]=d^{k+1}w
    KI = dp[C - 1::-1] * a64                        # [C, N] KI[j]=a d^{C-1-j}
    KFp = 1000.0 * dp[C - 2::-1] * b64              # [C-1,N] KFp[i]=1000b d^{C-2-i}
    KF0PAD = np.zeros((C, N), f64)
    KF0PAD[C - 1, :] = 1000.0 * dp[C - 1] * b64
    DIAGC = np.diag(dp[C])                          # [N, N] diag(d^C)
    DIAGD = np.diag(d)                              # [N, N] diag(d)

    # u-matmul block constant: one K=64 operand per group.  lhsT rows 0..31
    # hold I_t, rows 32..63 hold f_{t-1}; RAB[s, j*512 + nl*32 + tl] pairs
    # row s with output column (n0+nl, tl): a_n delta(s,tl) on the I half,
    # 1000 b_n delta(s-32,tl) on the f half.
    RAB = np.zeros((2 * GQ, 4096), f64)
    for j in range(8):
        for nl in range(16):
            nn = 16 * j + nl
            for s in range(GQ):
                RAB[s, j * 512 + nl * GQ + s] = a64[nn]
                RAB[GQ + s, j * 512 + nl * GQ + s] = 1000.0 * b64[nn]

    mrow = np.repeat(d, TT)
    mrow[0::TT] = 0.0
    DMASK = np.tile(mrow, (128, 1))                 # [128, N*TT]
    DREP = np.tile(d, (128, 1))                     # [128, N]
    COLS = np.tile(np.array([e[3], e[2], e[1], e[0]], f64), (128, 1))

    cst = dict(W2=W2, TIT=TIT, LT2=LT2, GAMPAD=GAMPAD, KI=KI, KFp=KFp,
               KF0PAD=KF0PAD, DIAGC=DIAGC, DIAGD=DIAGD, RAB=RAB,
               DMASK=DMASK, DREP=DREP, COLS=COLS)
    cst = {k: np.ascontiguousarray(v, np.float32) for k, v in cst.items()}
    meta = dict(e=e, dp=dp, gam=gam, kf0=1000.0 * dp[C - 1] * b64)
    return cst, meta


def _pick_ni(Is, cst, meta):
    """Rehearse the chunked Jacobi iteration (numpy fp32); pick the first
    sweep count where successive sweeps differ by < 1e-3 absolute, plus one."""
    f32 = np.float32
    e0, e1, e2, e3 = (f32(x) for x in meta["e"])

    def phi(z):
        s = z * z
        return np.maximum(f32(100.0) * np.tanh(z * (s * e3 + e1) + (s * e2 + e0),
                                               dtype=f32), f32(0.0))

    W2 = cst["W2"]; TIT = cst["TIT"]; LT2 = cst["LT2"]
    KI = cst["KI"]; KFp = cst["KFp"]
    gam = meta["gam"].astype(f32)
    dC = meta["dp"][C].astype(f32)
    kf0 = meta["kf0"].astype(f32)          # 1000 * b * d^{C-1}

    need = 3
    vbT = np.zeros((N, B), f32)
    fb = np.zeros((B,), f32)
    for ci in range(NCH):
        Cc = C if ci < NCH - 1 else CLAST
        I_ch = Is[ci * C: ci * C + Cc].astype(f32)
        Zc = (W2.T[:Cc] @ vbT + TIT[:Cc, :Cc].T @ I_ch
              + gam[:Cc][:, None] * fb[None, :]).astype(f32)
        Fx = np.zeros((Cc, B), f32)
        used = 12
        for it in range(12):
            Fn = phi(Zc + LT2[:Cc, :Cc].T @ Fx)
            delta = float(np.abs(Fn - Fx).max())
            Fx = Fn
            if delta < 5e-3:
                used = it + 1
                break
        need = max(need, used)
        if ci < NCH - 1:
            vbT = (dC[:, None] * vbT + KI.T @ I_ch + KFp.T @ Fx[:C - 1]
                   + kf0[:, None] * fb[None, :]).astype(f32)
            fb = Fx[Cc - 1]
    return int(min(max(need, 3), 12))


# --------------------------------------------------------------------------
# device program
# --------------------------------------------------------------------------

@functools.lru_cache(maxsize=4)
def _build(NI):
    import concourse.bacc as bacc
    import concourse.mybir as mybir
    from concourse.tile import TileContext

    f32 = mybir.dt.float32
    f32r = mybir.dt.float32r
    Op = mybir.AluOpType
    AF = mybir.ActivationFunctionType

    nc = bacc.Bacc(None, target_bir_lowering=False)

    def pin(name, shape, dt=f32):
        return nc.declare_dram_parameter(name, list(shape), dt, isOutput=False)

    IsS = pin("IsS", (T, BLOC))
    dW2 = pin("W2", (N, C), f32r); dTIT = pin("TIT", (C, C), f32r)
    dLT2 = pin("LT2", (C, C), f32r); dGAMPAD = pin("GAMPAD", (C, C), f32r)
    dKI = pin("KI", (C, N), f32r); dKFp = pin("KFp", (C - 1, N), f32r)
    dKF0PAD = pin("KF0PAD", (C, N), f32r); dDIAGC = pin("DIAGC", (N, N), f32r)
    dDIAGD = pin("DIAGD", (N, N), f32r)
    dRAB = pin("RAB", (2 * GQ, 4096), f32r)
    dDMASK = pin("DMASK", (128, N * TT)); dDREP = pin("DREP", (128, N))
    dCOLS = pin("COLS", (128, 4))
    fs_out = nc.declare_dram_parameter("fs_dev", [NCH, C, BLOC], f32, isOutput=True)
    vs_out = nc.declare_dram_parameter("vs_dev", [NTILES, 128, N * TT], f32,
                                       isOutput=True)

    with TileContext(nc) as tc:
        with (
            tc.tile_pool(name="const", bufs=1) as cpool,
            tc.tile_pool(name="uv", bufs=3) as uvpool,
            tc.tile_pool(name="small", bufs=1) as spool,
            tc.tile_pool(name="ch", bufs=2) as chpool,
            tc.tile_pool(name="zp", bufs=2, space="PSUM") as zpool,
            tc.tile_pool(name="vp", bufs=1, space="PSUM") as vppool,
            tc.tile_pool(name="up", bufs=2, space="PSUM") as upool,
        ):
            def cload(dram, shape, tag, dt=f32):
                t = cpool.tile(list(shape), dt, tag=tag, name=tag)
                nc.sync.dma_start(t[:], dram[:])
                return t

            W2 = cload(dW2, (N, C), "cW2", f32r)
            TIT = cload(dTIT, (C, C), "cTIT", f32r)
            LT2 = cload(dLT2, (C, C), "cLT2", f32r)
            GAMPAD = cload(dGAMPAD, (C, C), "cGAMPAD", f32r)
            KI = cload(dKI, (C, N), "cKI", f32r)
            KFp = cload(dKFp, (C - 1, N), "cKFp", f32r)
            KF0PAD = cload(dKF0PAD, (C, N), "cKF0PAD", f32r)
            DIAGC = cload(dDIAGC, (N, N), "cDIAGC", f32r)
            DIAGD = cload(dDIAGD, (N, N), "cDIAGD", f32r)
            RAB = cload(dRAB, (2 * GQ, 4096), "cRAB", f32r)
            DMASK = cload(dDMASK, (128, N * TT), "cDMASK")
            DREP = cload(dDREP, (128, N), "cDREP")
            COLS = cload(dCOLS, (128, 4), "cCOLS")

            vbs = [spool.tile([N, BLOC], f32r, tag="vba", name="vba"),
                   spool.tile([N, BLOC], f32r, tag="vbb", name="vbb")]
            nc.vector.memset(vbs[0][:].bitcast(f32), 0.0)

            Fxs = []
            Its = []

            def emit_chain_p1(ci):
                Cc = C if ci < NCH - 1 else CLAST
                t0 = ci * C
                It = spool.tile([128, BLOC], f32r, tag=f"it{ci}", name=f"it{ci}")
                nc.gpsimd.dma_start(It[0:Cc, :], IsS[t0:t0 + Cc, :].bitcast(f32r))
                Fx = spool.tile([128, BLOC], f32r, tag=f"fx{ci}", name=f"fx{ci}")
                nc.gpsimd.memset(Fx[:].bitcast(f32), 0.0)
                Fxs.append(Fx)
                Its.append(It)
                chain_sweeps(ci, [0])

            def emit_chain_p2(ci):
                chain_sweeps(ci, range(1, NI))
                chain_finish(ci)

            def chain_sweeps(ci, sweeps):
                Cc = C if ci < NCH - 1 else CLAST
                It = Its[ci]
                Fx = Fxs[ci]
                vbc = vbs[ci % 2]
                Fprev = Fxs[ci - 1] if ci > 0 else None

                for sw in sweeps:
                    z = zpool.tile([128, BLOC], f32, tag="z")
                    zs = z[0:Cc, :]
                    nc.tensor.matmul(zs, W2[:, 0:Cc], vbc[:], start=True, stop=False)
                    nc.tensor.matmul(zs, TIT[0:Cc, 0:Cc], It[0:Cc, :],
                                     start=False, stop=False)
                    if ci > 0:
                        nc.tensor.matmul(zs, GAMPAD[:, 0:Cc], Fprev[:],
                                         start=False, stop=False)
                    nc.tensor.matmul(zs, LT2[0:Cc, 0:Cc], Fx[0:Cc, :],
                                     start=False, stop=True)
                    S = chpool.tile([128, BLOC], f32, tag="S")
                    nc.scalar.activation(S[0:Cc, :], zs, AF.Square)
                    q = chpool.tile([128, BLOC], f32, tag="q")
                    nc.scalar.activation(q[0:Cc, :], S[0:Cc, :], AF.Identity,
                                         bias=COLS[0:Cc, 2:3], scale=COLS[0:Cc, 0:1])
                    m2 = chpool.tile([128, BLOC], f32, tag="m2")
                    nc.vector.scalar_tensor_tensor(m2[0:Cc, :], zs, 1.0, q[0:Cc, :],
                                                   op0=Op.mult, op1=Op.mult)
                    r = chpool.tile([128, BLOC], f32, tag="r")
                    nc.vector.scalar_tensor_tensor(r[0:Cc, :], S[0:Cc, :],
                                                   COLS[0:Cc, 1:2], m2[0:Cc, :],
                                                   op0=Op.mult, op1=Op.add)
                    Gt = chpool.tile([128, BLOC], f32, tag="G")
                    nc.scalar.activation(Gt[0:Cc, :], r[0:Cc, :], AF.Tanh,
                                         bias=COLS[0:Cc, 3:4])
                    nc.scalar.activation(Fx[0:Cc, :], Gt[0:Cc, :], AF.Relu,
                                         scale=100.0)

            def chain_finish(ci):
                Cc = C if ci < NCH - 1 else CLAST
                It = Its[ci]
                Fx = Fxs[ci]
                vbc = vbs[ci % 2]
                Fprev = Fxs[ci - 1] if ci > 0 else None

                # ---- chunk-end state carry ----
                if ci < NCH - 1:
                    ve = vppool.tile([N, BLOC], f32, tag="ve")
                    nc.tensor.matmul(ve[:], DIAGC[:], vbc[:], start=True, stop=False)
                    nc.tensor.matmul(ve[:], KI[:], It[:], start=False, stop=False)
                    if ci > 0:
                        nc.tensor.matmul(ve[:], KF0PAD[:], Fprev[:],
                                         start=False, stop=False)
                    nc.tensor.matmul(ve[:], KFp[:], Fx[0:C - 1, :], start=False,
                                     stop=True)
                    nc.scalar.copy(vbs[(ci + 1) % 2][:], ve[:])

                nc.gpsimd.dma_start(fs_out[ci, 0:Cc, :], Fx[0:Cc, :].bitcast(f32))

            def emit_bulk_prep(ci):
                Cc = C if ci < NCH - 1 else CLAST
                t0 = ci * C
                Fx = Fxs[ci]
                Fprev = Fxs[ci - 1] if ci > 0 else None
                vbc = vbs[ci % 2]

                # ---- combined u operands: rows 0..31 = I, rows 32..63 = f_(t-1)
                cmbs = []
                for g in range(4):
                    cmb = spool.tile([2 * GQ, BLOC], f32r, tag=f"cmb{g}_{ci % 3}",
                                     name=f"cmb{g}_{ci % 3}")
                    ni_ = min(GQ, max(Cc - GQ * g, 0))
                    if ni_ > 0:
                        nc.gpsimd.dma_start(
                            cmb[0:ni_, :],
                            IsS[t0 + GQ * g:t0 + GQ * g + ni_, :].bitcast(f32r))
                    if g == 0:
                        if ci == 0:
                            nc.vector.memset(cmb[GQ:GQ + 1, :].bitcast(f32), 0.0)
                        else:
                            nc.gpsimd.dma_start(cmb[GQ:GQ + 1, :],
                                                Fprev[C - 1:C, :])
                        nc.gpsimd.dma_start(cmb[GQ + 1:2 * GQ, :], Fx[0:GQ - 1, :])
                    else:
                        nc.gpsimd.dma_start(cmb[GQ:2 * GQ, :],
                                            Fx[GQ * g - 1:GQ * g + GQ - 1, :])
                    cmbs.append(cmb)
                state["cmbs"] = cmbs

                # d * v_base^T in [b, n] layout for the chunk-start injection
                if ci > 0:
                    vt = vppool.tile([128, N], f32, tag="vt")
                    nc.tensor.matmul(vt[:], vbc[:], DIAGD[:], start=True, stop=True)
                    vbTd = chpool.tile([128, N], f32, tag="vbTd")
                    nc.scalar.copy(vbTd[:], vt[:])
                    state["vbTd"] = vbTd

            def emit_bulk_tile(ci, h):
                cmbs = state["cmbs"]
                if True:
                    U = uvpool.tile([128, N * TT], f32, tag="uv")
                    U3 = U[:, :].rearrange("p (n t) -> p n t", n=N)
                    for g in (2 * h, 2 * h + 1):
                        tl0 = (g - 2 * h) * GQ          # 0 or 32 within tile
                        for pj in range(4):             # pairs of n-slices
                            ups = upool.tile([128, 1024], f32, tag="up")
                            for half in range(2):
                                j = 2 * pj + half
                                seg = ups[:, half * 512:half * 512 + 512]
                                nc.tensor.matmul(
                                    seg, cmbs[g][:],
                                    RAB[:, j * 512:(j + 1) * 512],
                                    start=True, stop=True)
                            src = ups[:, :].rearrange("p (n t) -> p n t", n=GQ)
                            nc.scalar.copy(
                                U3[:, 32 * pj:32 * pj + 32, tl0:tl0 + GQ], src)
                    if h == 0:
                        if ci > 0:
                            nc.vector.tensor_tensor(U[:, 0::TT], U[:, 0::TT],
                                                    state["vbTd"][:], Op.add)
                    else:
                        tmp = chpool.tile([128, N], f32, tag="inj")
                        nc.vector.tensor_tensor(tmp[:], state["Uprev"][:, TT - 1::TT],
                                                DREP[:], Op.mult)
                        nc.vector.tensor_tensor(U[:, 0::TT], U[:, 0::TT], tmp[:],
                                                Op.add)
                    nc.vector.tensor_tensor_scan(U[:, :], DMASK[:, :], U[:, :],
                                                 initial=0.0, op0=Op.mult,
                                                 op1=Op.add)
                    nc.sync.dma_start(vs_out[2 * ci + h, :, :], U[:, :])
                    state["Uprev"] = U

            state = {}
            emit_chain_p1(0)
            emit_chain_p2(0)
            for ci in range(NCH):
                emit_bulk_prep(ci)
                if ci + 1 < NCH:
                    emit_chain_p1(ci + 1)
                emit_bulk_tile(ci, 0)
                if ci + 1 < NCH:
                    emit_chain_p2(ci + 1)
                emit_bulk_tile(ci, 1)

    nc.compile()
    return nc


# --------------------------------------------------------------------------
# entry point
# --------------------------------------------------------------------------

def _ensure_axon_hooks_shim():
    """bass_utils' axon trace path imports antenv.axon_hooks, which may be
    missing from this image; give it a settable no-op shim so trace=True
    degrades gracefully instead of raising."""
    import sys
    import types
    try:
        import antenv.axon_hooks  # noqa: F401
        return sys.modules["antenv.axon_hooks"]
    except ImportError:
        pass
    import antenv
    mod = types.ModuleType("antenv.axon_hooks")
    mod._hook = None
    mod.set_axon_ntff_profile_hook = lambda h: setattr(mod, "_hook", h)
    mod.get_axon_ntff_profile_hook = lambda: mod._hook
    sys.modules["antenv.axon_hooks"] = mod
    antenv.axon_hooks = mod
    return mod


def kernel(Is, ds, a, b, w, poly_coeff, g_b):
    global _last_results
    import os
    from concourse.bass_utils import run_bass_kernel_spmd

    _ensure_axon_hooks_shim()

    Is = np.ascontiguousarray(np.asarray(Is, np.float32))
    cst, meta = _host_constants(np.asarray(ds), np.asarray(a), np.asarray(b),
                                np.asarray(w), np.asarray(poly_coeff),
                                np.asarray(g_b))
    NI = _pick_ni(Is, cst, meta)

    nc = _build(NI)

    in_maps = []
    for c in range(NCORES):
        m = dict(cst)
        m["IsS"] = np.ascontiguousarray(Is[:, c * BLOC:(c + 1) * BLOC])
        in_maps.append(m)

    trace = os.environ.get("BASS_TRACE", "").lower() in ("1", "true", "yes")
    res = run_bass_kernel_spmd(nc, in_maps, list(range(NCORES)), trace=trace)
    _last_results = res

    fs = np.empty((T, B), np.float32)
    vs = np.empty((T, B, N), np.float32)
    for c in range(NCORES):
        out = res.results[c]
        fsd = out["fs_dev"]
        vsd = out["vs_dev"].reshape(NTILES, 128, N, TT)
        b0 = c * BLOC
        for ci in range(NCH):
            Cc = C if ci < NCH - 1 else CLAST
            fs[ci * C:ci * C + Cc, b0:b0 + BLOC] = fsd[ci, :Cc]
            for h in range(2):
                ntt = min(TT, Cc - h * TT)
                if ntt <= 0:
                    continue
                tt0 = ci * C + h * TT
                vs[tt0:tt0 + ntt, b0:b0 + BLOC, :] = (
                    vsd[2 * ci + h, :, :, :ntt].transpose(2, 0, 1))
    return fs, vs


# revision 16
# speedup vs baseline: 1.2113x; 1.0203x over previous
"""Trainium2 Bass kernel for ExponentialKernelFiringRateModel (batched predict).

Model (reference semantics, fp32):
    v_t = (1-ds)*v_{t-1} + outer(I_t, a) + 1000*outer(f_{t-1}, b)      [B, N]
    z_t = v_t @ w ;  f_t = relu(100*tanh(poly((z_t - g_b)/1000)))      [B]
returns (pred_fs [T,B], vs [T,B,N]).

Strategy (8 cores, data-parallel over batch, 128 batch elements per core):
  The T=1000 serial scan is cut into 8 chunks of C=128 steps. Within a chunk
  z is affine in the chunk-local f history with weak coupling (|gamma|~0.01-
  0.1), so the nonlinear fixed point F = phi(Zbase + L F) is solved by a few
  Jacobi sweeps of tensor-engine matmuls ([C,C] Toeplitz operators); the
  sweep count NI comes from a fast host-side numpy rehearsal. The chunk-end
  state v_end (the only serial carry) is produced directly by three more
  matmuls, so the serial path never waits for the bulk pipeline. The bulk
  output vs is materialized per 64-step tile: tensor-engine matmuls expand
  u_t = a*I_t + 1000*b*f_{t-1} against block-diagonal constants (K=32 groups
  at legal PE quadrant bases), then one DVE `tensor_tensor_scan` per tile
  runs the IIR v = d*v + u in-place along the time axis (segment restarts
  via a zero-masked decay operand), and the result streams to HBM as flat
  contiguous tiles that the host reorders into [T,B,N].
"""

import functools
import numpy as np

T, B, N = 1000, 1024, 128
NCORES = 8
BLOC = B // NCORES          # 128 batch per core
C = 128                     # chunk length; last chunk is 104
NCH = 8
CLAST = T - (NCH - 1) * C   # 104
TT = 64                     # scan-tile length (2 tiles per chunk)
NTILES = NCH * 2
GQ = 32                     # u-matmul contraction group (PE quadrant size)

_last_results = None


# --------------------------------------------------------------------------
# host-side precompute (float64 -> float32 tiles)
# --------------------------------------------------------------------------

def _host_constants(ds, a, b, w, poly_coeff, g_b):
    f64 = np.float64
    d = (1.0 - ds).astype(f64)
    a64 = a.astype(f64)
    b64 = b.astype(f64)
    wf = w.reshape(-1).astype(f64)
    c2 = poly_coeff.astype(f64) ** 2
    gb = float(np.asarray(g_b))

    # cubic in raw z: p(z) = sum_k c2[k] ((z-gb)/1000)^k = e0+e1 z+e2 z^2+e3 z^3
    e = np.zeros(4, f64)
    powc = np.array([1.0])
    for k in range(4):
        e[: len(powc)] += c2[k] * powc
        powc = np.convolve(powc, [-gb / 1000.0, 1.0 / 1000.0])

    dp = np.empty((C + 1, N), f64)
    dp[0] = 1.0
    for m in range(1, C + 1):
        dp[m] = dp[m - 1] * d

    dw = dp * wf                      # dw[m, n] = d^m w
    tau = dw[:C] @ a64                # tau[m] = a . D^m w
    gam = 1000.0 * (dw[:C] @ b64)     # gam[m] = 1000 b . D^m w

    idx = np.arange(C)
    MK = idx[None, :] - idx[:, None]               # MK[j, k] = k - j
    TIT = np.where(MK >= 0, tau[np.clip(MK, 0, C - 1)], 0.0)      # [j, k]
    LT2 = np.where(MK >= 1, gam[np.clip(MK - 1, 0, C - 1)], 0.0)  # [i, k]
    GAMPAD = np.zeros((C, C), f64)
    GAMPAD[C - 1, :] = gam                          # row 127 -> gamma_k

    W2 = (dp[1:C + 1] * wf).T                       # [N, C] W2[n,k]=d^{k+1}w
    KI = dp[C - 1::-1] * a64                        # [C, N] KI[j]=a d^{C-1-j}
    KFp = 1000.0 * dp[C - 2::-1] * b64              # [C-1,N] KFp[i]=1000b d^{C-2-i}
    KF0PAD = np.zeros((C, N), f64)
    KF0PAD[C - 1, :] = 1000.0 * dp[C - 1] * b64
    DIAGC = np.diag(dp[C])                          # [N, N] diag(d^C)
    DIAGD = np.diag(d)                              # [N, N] diag(d)

    # u-matmul block constant: one K=64 operand per group.  lhsT rows 0..31
    # hold I_t, rows 32..63 hold f_{t-1}; RAB[s, j*512 + nl*32 + tl] pairs
    # row s with output column (n0+nl, tl): a_n delta(s,tl) on the I half,
    # 1000 b_n delta(s-32,tl) on the f half.
    RAB = np.zeros((2 * GQ, 4096), f64)
    for j in range(8):
        for nl in range(16):
            nn = 16 * j + nl
            for s in range(GQ):
                RAB[s, j * 512 + nl * GQ + s] = a64[nn]
                RAB[GQ + s, j * 512 + nl * GQ + s] = 1000.0 * b64[nn]

    mrow = np.repeat(d, TT)
    mrow[0::TT] = 0.0
    DMASK = np.tile(mrow, (128, 1))                 # [128, N*TT]
    DREP = np.tile(d, (128, 1))                     # [128, N]
    COLS = np.tile(np.array([e[3], e[2], e[1], e[0]], f64), (128, 1))

    cst = dict(W2=W2, TIT=TIT, LT2=LT2, GAMPAD=GAMPAD, KI=KI, KFp=KFp,
               KF0PAD=KF0PAD, DIAGC=DIAGC, DIAGD=DIAGD, RAB=RAB,
               DMASK=DMASK, DREP=DREP, COLS=COLS)
    cst = {k: np.ascontiguousarray(v, np.float32) for k, v in cst.items()}
    meta = dict(e=e, dp=dp, gam=gam, kf0=1000.0 * dp[C - 1] * b64)
    return cst, meta


def _pick_ni(Is, cst, meta):
    """Rehearse the chunked Jacobi iteration (numpy fp32); pick the first
    sweep count where successive sweeps differ by < 1e-3 absolute, plus one."""
    f32 = np.float32
    e0, e1, e2, e3 = (f32(x) for x in meta["e"])

    def phi(z):
        s = z * z
        return np.maximum(f32(100.0) * np.tanh(z * (s * e3 + e1) + (s * e2 + e0),
                                               dtype=f32), f32(0.0))

    W2 = cst["W2"]; TIT = cst["TIT"]; LT2 = cst["LT2"]
    KI = cst["KI"]; KFp = cst["KFp"]
    gam = meta["gam"].astype(f32)
    dC = meta["dp"][C].astype(f32)
    kf0 = meta["kf0"].astype(f32)          # 1000 * b * d^{C-1}

    need = 3
    vbT = np.zeros((N, B), f32)
    fb = np.zeros((B,), f32)
    for ci in range(NCH):
        Cc = C if ci < NCH - 1 else CLAST
        I_ch = Is[ci * C: ci * C + Cc].astype(f32)
        Zc = (W2.T[:Cc] @ vbT + TIT[:Cc, :Cc].T @ I_ch
              + gam[:Cc][:, None] * fb[None, :]).astype(f32)
        Fx = np.zeros((Cc, B), f32)
        used = 12
        for it in range(12):
            Fn = phi(Zc + LT2[:Cc, :Cc].T @ Fx)
            delta = float(np.abs(Fn - Fx).max())
            Fx = Fn
            if delta < 5e-3:
                used = it + 1
                break
        need = max(need, used)
        if ci < NCH - 1:
            vbT = (dC[:, None] * vbT + KI.T @ I_ch + KFp.T @ Fx[:C - 1]
                   + kf0[:, None] * fb[None, :]).astype(f32)
            fb = Fx[Cc - 1]
    return int(min(max(need, 3), 12))


# --------------------------------------------------------------------------
# device program
# --------------------------------------------------------------------------

@functools.lru_cache(maxsize=4)
def _build(NI):
    import concourse.bacc as bacc
    import concourse.mybir as mybir
    from concourse.tile import TileContext

    f32 = mybir.dt.float32
    f32r = mybir.dt.float32r
    Op = mybir.AluOpType
    AF = mybir.ActivationFunctionType

    nc = bacc.Bacc(None, target_bir_lowering=False)

    def pin(name, shape, dt=f32):
        return nc.declare_dram_parameter(name, list(shape), dt, isOutput=False)

    IsS = pin("IsS", (T, BLOC))
    dW2 = pin("W2", (N, C), f32r); dTIT = pin("TIT", (C, C), f32r)
    dLT2 = pin("LT2", (C, C), f32r); dGAMPAD = pin("GAMPAD", (C, C), f32r)
    dKI = pin("KI", (C, N), f32r); dKFp = pin("KFp", (C - 1, N), f32r)
    dKF0PAD = pin("KF0PAD", (C, N), f32r); dDIAGC = pin("DIAGC", (N, N), f32r)
    dDIAGD = pin("DIAGD", (N, N), f32r)
    dRAB = pin("RAB", (2 * GQ, 4096), f32r)
    dDMASK = pin("DMASK", (128, N * TT)); dDREP = pin("DREP", (128, N))
    dCOLS = pin("COLS", (128, 4))
    fs_out = nc.declare_dram_parameter("fs_dev", [NCH, C, BLOC], f32, isOutput=True)
    vs_out = nc.declare_dram_parameter("vs_dev", [NTILES, 128, N * TT], f32,
                                       isOutput=True)

    with TileContext(nc) as tc:
        with (
            tc.tile_pool(name="const", bufs=1) as cpool,
            tc.tile_pool(name="uv", bufs=3) as uvpool,
            tc.tile_pool(name="small", bufs=1) as spool,
            tc.tile_pool(name="ch", bufs=2) as chpool,
            tc.tile_pool(name="zp", bufs=2, space="PSUM") as zpool,
            tc.tile_pool(name="vp", bufs=1, space="PSUM") as vppool,
            tc.tile_pool(name="up", bufs=2, space="PSUM") as upool,
        ):
            def cload(dram, shape, tag, dt=f32):
                t = cpool.tile(list(shape), dt, tag=tag, name=tag)
                nc.sync.dma_start(t[:], dram[:])
                return t

            W2 = cload(dW2, (N, C), "cW2", f32r)
            TIT = cload(dTIT, (C, C), "cTIT", f32r)
            LT2 = cload(dLT2, (C, C), "cLT2", f32r)
            GAMPAD = cload(dGAMPAD, (C, C), "cGAMPAD", f32r)
            COLS = cload(dCOLS, (128, 4), "cCOLS")
            RAB = cload(dRAB, (2 * GQ, 4096), "cRAB", f32r)
            DMASK = cload(dDMASK, (128, N * TT), "cDMASK")
            KI = cload(dKI, (C, N), "cKI", f32r)
            KFp = cload(dKFp, (C - 1, N), "cKFp", f32r)
            KF0PAD = cload(dKF0PAD, (C, N), "cKF0PAD", f32r)
            DIAGC = cload(dDIAGC, (N, N), "cDIAGC", f32r)
            DIAGD = cload(dDIAGD, (N, N), "cDIAGD", f32r)
            DREP = cload(dDREP, (128, N), "cDREP")

            vbs = [spool.tile([N, BLOC], f32r, tag="vba", name="vba"),
                   spool.tile([N, BLOC], f32r, tag="vbb", name="vbb")]
            nc.vector.memset(vbs[0][:].bitcast(f32), 0.0)

            Fxs = []
            Its = []

            def emit_chain_p1(ci):
                Cc = C if ci < NCH - 1 else CLAST
                t0 = ci * C
                It = spool.tile([128, BLOC], f32r, tag=f"it{ci}", name=f"it{ci}")
                nc.gpsimd.dma_start(It[0:Cc, :], IsS[t0:t0 + Cc, :].bitcast(f32r))
                Fx = spool.tile([128, BLOC], f32r, tag=f"fx{ci}", name=f"fx{ci}")
                nc.gpsimd.memset(Fx[:].bitcast(f32), 0.0)
                Fxs.append(Fx)
                Its.append(It)
                chain_sweeps(ci, [0])

            def emit_chain_p2(ci):
                chain_sweeps(ci, range(1, NI))
                chain_finish(ci)

            def chain_sweeps(ci, sweeps):
                Cc = C if ci < NCH - 1 else CLAST
                It = Its[ci]
                Fx = Fxs[ci]
                vbc = vbs[ci % 2]
                Fprev = Fxs[ci - 1] if ci > 0 else None

                for sw in sweeps:
                    z = zpool.tile([128, BLOC], f32, tag="z")
                    zs = z[0:Cc, :]
                    nc.tensor.matmul(zs, W2[:, 0:Cc], vbc[:], start=True, stop=False)
                    nc.tensor.matmul(zs, TIT[0:Cc, 0:Cc], It[0:Cc, :],
                                     start=False, stop=False)
                    if ci > 0:
                        nc.tensor.matmul(zs, GAMPAD[:, 0:Cc], Fprev[:],
                                         start=False, stop=False)
                    nc.tensor.matmul(zs, LT2[0:Cc, 0:Cc], Fx[0:Cc, :],
                                     start=False, stop=True)
                    S = chpool.tile([128, BLOC], f32, tag="S")
                    nc.scalar.activation(S[0:Cc, :], zs, AF.Square)
                    q = chpool.tile([128, BLOC], f32, tag="q")
                    nc.scalar.activation(q[0:Cc, :], S[0:Cc, :], AF.Identity,
                                         bias=COLS[0:Cc, 2:3], scale=COLS[0:Cc, 0:1])
                    m2 = chpool.tile([128, BLOC], f32, tag="m2")
                    nc.vector.scalar_tensor_tensor(m2[0:Cc, :], zs, 1.0, q[0:Cc, :],
                                                   op0=Op.mult, op1=Op.mult)
                    r = chpool.tile([128, BLOC], f32, tag="r")
                    nc.vector.scalar_tensor_tensor(r[0:Cc, :], S[0:Cc, :],
                                                   COLS[0:Cc, 1:2], m2[0:Cc, :],
                                                   op0=Op.mult, op1=Op.add)
                    Gt = chpool.tile([128, BLOC], f32, tag="G")
                    nc.scalar.activation(Gt[0:Cc, :], r[0:Cc, :], AF.Tanh,
                                         bias=COLS[0:Cc, 3:4])
                    nc.scalar.activation(Fx[0:Cc, :], Gt[0:Cc, :], AF.Relu,
                                         scale=100.0)

            def chain_finish(ci):
                Cc = C if ci < NCH - 1 else CLAST
                It = Its[ci]
                Fx = Fxs[ci]
                vbc = vbs[ci % 2]
                Fprev = Fxs[ci - 1] if ci > 0 else None

                # ---- chunk-end state carry ----
                if ci < NCH - 1:
                    ve = vppool.tile([N, BLOC], f32, tag="ve")
                    nc.tensor.matmul(ve[:], DIAGC[:], vbc[:], start=True, stop=False)
                    nc.tensor.matmul(ve[:], KI[:], It[:], start=False, stop=False)
                    if ci > 0:
                        nc.tensor.matmul(ve[:], KF0PAD[:], Fprev[:],
                                         start=False, stop=False)
                    nc.tensor.matmul(ve[:], KFp[:], Fx[0:C - 1, :], start=False,
                                     stop=True)
                    nc.scalar.copy(vbs[(ci + 1) % 2][:], ve[:])

                nc.gpsimd.dma_start(fs_out[ci, 0:Cc, :], Fx[0:Cc, :].bitcast(f32))

            def emit_bulk_prep(ci):
                Cc = C if ci < NCH - 1 else CLAST
                t0 = ci * C
                Fx = Fxs[ci]
                Fprev = Fxs[ci - 1] if ci > 0 else None
                vbc = vbs[ci % 2]

                # ---- combined u operands: rows 0..31 = I, rows 32..63 = f_(t-1)
                cmbs = []
                for g in range(4):
                    cmb = spool.tile([2 * GQ, BLOC], f32r, tag=f"cmb{g}_{ci % 3}",
                                     name=f"cmb{g}_{ci % 3}")
                    ni_ = min(GQ, max(Cc - GQ * g, 0))
                    if ni_ > 0:
                        nc.gpsimd.dma_start(
                            cmb[0:ni_, :],
                            IsS[t0 + GQ * g:t0 + GQ * g + ni_, :].bitcast(f32r))
                    if g == 0:
                        if ci == 0:
                            nc.vector.memset(cmb[GQ:GQ + 1, :].bitcast(f32), 0.0)
                        else:
                            nc.gpsimd.dma_start(cmb[GQ:GQ + 1, :],
                                                Fprev[C - 1:C, :])
                        nc.gpsimd.dma_start(cmb[GQ + 1:2 * GQ, :], Fx[0:GQ - 1, :])
                    else:
                        nc.gpsimd.dma_start(cmb[GQ:2 * GQ, :],
                                            Fx[GQ * g - 1:GQ * g + GQ - 1, :])
                    cmbs.append(cmb)
                state[("cmbs", ci)] = cmbs

                # d * v_base^T in [b, n] layout for the chunk-start injection
                if ci > 0:
                    vt = vppool.tile([128, N], f32, tag="vt")
                    nc.tensor.matmul(vt[:], vbc[:], DIAGD[:], start=True, stop=True)
                    vbTd = chpool.tile([128, N], f32, tag="vbTd")
                    nc.scalar.copy(vbTd[:], vt[:])
                    state[("vbTd", ci)] = vbTd

            def emit_bulk_tile(ci, h):
                cmbs = state[("cmbs", ci)]
                if True:
                    U = uvpool.tile([128, N * TT], f32, tag="uv")
                    U3 = U[:, :].rearrange("p (n t) -> p n t", n=N)
                    for g in (2 * h, 2 * h + 1):
                        tl0 = (g - 2 * h) * GQ          # 0 or 32 within tile
                        for pj in range(4):             # pairs of n-slices
                            ups = upool.tile([128, 1024], f32, tag="up")
                            for half in range(2):
                                j = 2 * pj + half
                                seg = ups[:, half * 512:half * 512 + 512]
                                nc.tensor.matmul(
                                    seg, cmbs[g][:],
                                    RAB[:, j * 512:(j + 1) * 512],
                                    start=True, stop=True)
                            src = ups[:, :].rearrange("p (n t) -> p n t", n=GQ)
                            nc.scalar.copy(
                                U3[:, 32 * pj:32 * pj + 32, tl0:tl0 + GQ], src)
                    if h == 0:
                        if ci > 0:
                            nc.vector.tensor_tensor(U[:, 0::TT], U[:, 0::TT],
                                                    state[("vbTd", ci)][:], Op.add)
                    else:
                        tmp = chpool.tile([128, N], f32, tag="inj")
                        nc.vector.tensor_tensor(tmp[:], state[("Uprev", ci)][:, TT - 1::TT],
                                                DREP[:], Op.mult)
                        nc.vector.tensor_tensor(U[:, 0::TT], U[:, 0::TT], tmp[:],
                                                Op.add)
                    nc.vector.tensor_tensor_scan(U[:, :], DMASK[:, :], U[:, :],
                                                 initial=0.0, op0=Op.mult,
                                                 op1=Op.add)
                    nc.sync.dma_start(vs_out[2 * ci + h, :, :], U[:, :])
                    state[("Uprev", ci)] = U

            state = {}
            emit_chain_p1(0)
            emit_chain_p2(0)
            emit_bulk_prep(0)
            for ci in range(NCH):
                if ci + 1 < NCH:
                    emit_chain_p1(ci + 1)
                emit_bulk_tile(ci, 0)
                if ci + 1 < NCH:
                    emit_chain_p2(ci + 1)
                    emit_bulk_prep(ci + 1)
                emit_bulk_tile(ci, 1)

    nc.compile()
    return nc


# --------------------------------------------------------------------------
# entry point
# --------------------------------------------------------------------------

def _ensure_axon_hooks_shim():
    """bass_utils' axon trace path imports antenv.axon_hooks, which may be
    missing from this image; give it a settable no-op shim so trace=True
    degrades gracefully instead of raising."""
    import sys
    import types
    try:
        import antenv.axon_hooks  # noqa: F401
        return sys.modules["antenv.axon_hooks"]
    except ImportError:
        pass
    import antenv
    mod = types.ModuleType("antenv.axon_hooks")
    mod._hook = None
    mod.set_axon_ntff_profile_hook = lambda h: setattr(mod, "_hook", h)
    mod.get_axon_ntff_profile_hook = lambda: mod._hook
    sys.modules["antenv.axon_hooks"] = mod
    antenv.axon_hooks = mod
    return mod


def kernel(Is, ds, a, b, w, poly_coeff, g_b):
    global _last_results
    import os
    from concourse.bass_utils import run_bass_kernel_spmd

    _ensure_axon_hooks_shim()

    Is = np.ascontiguousarray(np.asarray(Is, np.float32))
    cst, meta = _host_constants(np.asarray(ds), np.asarray(a), np.asarray(b),
                                np.asarray(w), np.asarray(poly_coeff),
                                np.asarray(g_b))
    NI = _pick_ni(Is, cst, meta)

    nc = _build(NI)

    in_maps = []
    for c in range(NCORES):
        m = dict(cst)
        m["IsS"] = np.ascontiguousarray(Is[:, c * BLOC:(c + 1) * BLOC])
        in_maps.append(m)

    trace = os.environ.get("BASS_TRACE", "").lower() in ("1", "true", "yes")
    res = run_bass_kernel_spmd(nc, in_maps, list(range(NCORES)), trace=trace)
    _last_results = res

    fs = np.empty((T, B), np.float32)
    vs = np.empty((T, B, N), np.float32)
    for c in range(NCORES):
        out = res.results[c]
        fsd = out["fs_dev"]
        vsd = out["vs_dev"].reshape(NTILES, 128, N, TT)
        b0 = c * BLOC
        for ci in range(NCH):
            Cc = C if ci < NCH - 1 else CLAST
            fs[ci * C:ci * C + Cc, b0:b0 + BLOC] = fsd[ci, :Cc]
            for h in range(2):
                ntt = min(TT, Cc - h * TT)
                if ntt <= 0:
                    continue
                tt0 = ci * C + h * TT
                vs[tt0:tt0 + ntt, b0:b0 + BLOC, :] = (
                    vsd[2 * ci + h, :, :, :ntt].transpose(2, 0, 1))
    return fs, vs
